# revision 29
# baseline (speedup 1.0000x reference)
"""Trainium2 Bass kernel for MinimalLinearAttention.

  q = relu(x @ q_w.T + q_b); k = relu(x @ k_w.T + k_b); v = x @ v_w.T + v_b
  kv[b,h] = sum_s k[b,s,h,:] outer v[b,s,h,:]          (per batch, all tokens)
  out[b,s,h] = q[b,s,h,:] @ kv[b,h]
  y = out @ o_w.T + o_b

Sharding: token-parallel over 8 cores. Each core takes a 512-token slice of
every batch (2048 tokens), computes k/v projections + partial kv, AllReduces
kv across cores (per batch), then does the q readout + output projection for
its own tokens. Host concatenates slices.

All-bf16 design, measured ~305us on HW (baseline 484us):
- Matmul operands bf16 so LDWEIGHTS (~105ns, FWL) hides under the ~263ns
  per-512-col matmul issue period (f32r loads take ~218ns and serialize).
  All weights stay resident in SBUF (12MB with x), so there is no
  stage-transition weight-load bubble.
- Input DMAs all on the sync queue (one HWDGE ring sustains ~360GB/s;
  splitting across two rings halves per-ring throughput), every transfer a
  contiguous DRAM block, ordered by first use: (wk[dn], x quarter-0 [dn])
  interleaved so the first K matmuls start at ~11us, then biases, wv,
  x q1, wq|wo merged per-din, x q2, q3.
- No bias matmuls: V/O biases ride the PSUM eviction as DVE tensor_tensor
  adds against host-replicated rows; K's bias is a DVE add then scalar-
  engine relu; Q bias is a per-partition activation scalar.
- kv pairs pack [128,128] blocks 4-to-a-bank (2 PSUM banks per batch),
  leaving 6 banks for projection-group rotation.
- Stage 2 hides the last kv AllReduce (~25us end-to-end; bounce-buffer
  reads wait on ALL collectives' semaphore): three batches of Q-projections
  (~50us of kv-independent PE work) run before the first readout, and
  readouts run long after their qt eviction so the PE never stalls on it.
- y stored bf16 in DRAM-contiguous [128,512] halves right after eviction.

On-device layouts (per core):
  xt   = x_slice.T            [4, D, 512] quarters (quarter = batch)
  w*   = W.T                  [Din=1024, Dout=1024] (wq|wo merged [D, 2048])
  K, V                        [T, D]     (from xt-stationary matmuls)
  Q^T                         [D, T]     (from w-stationary matmuls)
  kv per (batch, head-pair)   [128, 128] block-diagonal (2 heads of 64)
  y                           [NT, 2, 128, 512] bf16 (host reassembles f32)
"""

import os
import sys

os.environ.setdefault("MYCRO_LOCAL_CACHE", "1")

for _p in ("/opt/trn_rl_repo", "/root/.axon_site/_ro/trn_rl_repo"):
    if os.path.isdir(_p) and _p not in sys.path:
        sys.path.insert(0, _p)

import numpy as np

B, S, D, H, HD = 4, 4096, 1024, 16, 64
NCORES = 8
SC = S // NCORES          # 512 tokens per core per batch
T = B * SC                # 2048 rows per core
NPAIR = 8                 # head pairs (2 heads of 64 dims = 128 partitions)
NDIN = D // 128           # 8 Din tiles
NT = T // 128             # 16 T tiles per core
NTB = SC // 128           # 4 T tiles per batch

CC_BF16 = True            # bf16 kv collective payload

_CACHE = {}


def build_program_v6():
    if "nc_v6" in _CACHE:
        return _CACHE["nc_v6"]

    import concourse.bacc as bacc
    import concourse.tile as tile
    from concourse import bass, mybir

    f32 = mybir.dt.float32
    BF = mybir.dt.bfloat16
    CCDT = BF if CC_BF16 else f32
    RELU = mybir.ActivationFunctionType.Relu
    COPY = mybir.ActivationFunctionType.Copy
    ADD = mybir.AluOpType.add

    nc = bacc.Bacc("TRN2", target_bir_lowering=False, debug=False,
                   num_devices=NCORES)

    xt_d = nc.dram_tensor("xt", [4, D, 512], BF, kind="ExternalInput").ap()
    wk_d = nc.dram_tensor("wk", [D, D], BF, kind="ExternalInput").ap()
    wv_d = nc.dram_tensor("wv", [D, D], BF, kind="ExternalInput").ap()
    wqo_d = nc.dram_tensor("wqo", [D, 2 * D], BF, kind="ExternalInput").ap()
    bq_d = nc.dram_tensor("bq", [128, NDIN], f32, kind="ExternalInput").ap()
    # bkvo = [bkr | bvr | bor], each [128, D] replicated rows
    bkvo_d = nc.dram_tensor("bkvo", [128, 3 * D], BF,
                            kind="ExternalInput").ap()
    y_d = nc.dram_tensor("y", [NT, 2, 128, 512], BF,
                         kind="ExternalOutput").ap()

    HPB = 16 * 64  # bounce rows per batch: 16 heads x 64 d-rows

    with tile.TileContext(nc) as tc:
        with (
            tc.tile_pool(name="const", bufs=1) as constp,
            tc.tile_pool(name="wp", bufs=1) as wp,
            tc.tile_pool(name="xtp", bufs=1) as xtp,
            tc.tile_pool(name="kvb", bufs=6) as kvbp,
            tc.tile_pool(name="ktmp", bufs=3) as ktmpp,
            tc.tile_pool(name="qt", bufs=24) as qtp,
            tc.tile_pool(name="otb", bufs=10) as otbp,
            tc.tile_pool(name="kvex", bufs=8) as kvexp,
            tc.tile_pool(name="kvsb", bufs=24) as kvsbp,
            tc.tile_pool(name="yt", bufs=4) as ytp,
            tc.tile_pool(name="dram", bufs=1, space="DRAM") as dramp,
            tc.tile_pool(name="ps", bufs=6, space="PSUM") as psp,
            tc.tile_pool(name="pskv", bufs=2, space="PSUM") as pskvp,
        ):
            # ---- loads: first-use order, all on the sync queue (a single
            # HWDGE ring sustains ~360GB/s; splitting across two rings
            # halves per-ring throughput) ----
            qs = [nc.sync, nc.scalar]

            def dma(dst, src):
                nc.sync.dma_start(dst, src)

            wk_sb = [wp.tile([128, D], BF, tag=f"wk{dn}", name=f"wk_sb{dn}")
                     for dn in range(NDIN)]
            wv_sb = [wp.tile([128, D], BF, tag=f"wv{dn}", name=f"wv_sb{dn}")
                     for dn in range(NDIN)]
            xts = [xtp.tile([128, T], BF, tag=f"xt{dn}", name=f"xt_sb{dn}")
                   for dn in range(NDIN)]
            bkvo_sb = constp.tile([128, 3 * D], BF, tag="bkvo")
            bkr_sb = bkvo_sb[:, 0:D]
            bvr_sb = bkvo_sb[:, D:2 * D]
            bor_sb = bkvo_sb[:, 2 * D:3 * D]

            def xq(dn, q):
                return (xts[dn][:, q * 512:(q + 1) * 512],
                        xt_d[q, dn * 128:(dn + 1) * 128, :])

            # interleave wk[dn] with its x chunk so matmul dn of the first
            # K group starts as soon as its own operands land
            for dn in range(NDIN):
                dma(wk_sb[dn][:], wk_d[dn * 128:(dn + 1) * 128, :])
                dma(*xq(dn, 0))
            # biases early: the K eviction adds bkr; PSUM recycling depends
            # on those evictions
            dma(bkvo_sb[:], bkvo_d[:])
            for dn in range(NDIN):
                dma(wv_sb[dn][:], wv_d[dn * 128:(dn + 1) * 128, :])
            for dn in range(NDIN):
                dma(*xq(dn, 1))
            wqo_sb = [wp.tile([128, 2 * D], BF, tag=f"wqo{dn}",
                              name=f"wqo_sb{dn}") for dn in range(NDIN)]
            for dn in range(NDIN):
                dma(wqo_sb[dn][:], wqo_d[dn * 128:(dn + 1) * 128, :])
            for dn in range(NDIN):
                dma(*xq(dn, 2))
            for dn in range(NDIN):
                dma(*xq(dn, 3))
            wq_sb = [wqo_sb[dn][:, 0:D] for dn in range(NDIN)]
            wo_sb = [wqo_sb[dn][:, D:2 * D] for dn in range(NDIN)]
            bq_sb = constp.tile([128, NDIN], f32, tag="bq")
            dma(bq_sb[:], bq_d[:])

            bnc_in = [dramp.tile([HPB, 64], CCDT, tag=f"bi{b}",
                                 name=f"bnc_in{b}") for b in range(B)]
            bnc_out = [dramp.tile([HPB, 64], CCDT, tag=f"bo{b}",
                                  addr_space="Shared", name=f"bnc_out{b}")
                       for b in range(B)]

            # ---- Stage 1: K,V projections + per-batch partial kv ----
            def kproj(gt):
                kt = kvbp.tile([128, D], BF, tag="kb", name=f"kt{gt}")
                for hf in range(2):
                    ps = psp.tile([128, 512], f32, tag="ps")
                    for dn in range(NDIN):
                        nc.tensor.matmul(
                            ps[:],
                            xts[dn][:, gt * 128:(gt + 1) * 128],
                            wk_sb[dn][:, hf * 512:(hf + 1) * 512],
                            start=(dn == 0), stop=(dn == NDIN - 1))
                    ktmp = ktmpp.tile([128, 512], BF, tag="ktmp")
                    nc.vector.tensor_tensor(
                        ktmp[:], ps[:],
                        bkr_sb[:, hf * 512:(hf + 1) * 512], ADD)
                    nc.scalar.activation(
                        kt[:, hf * 512:(hf + 1) * 512], ktmp[:], RELU)
                return kt

            def vproj(gt):
                vt = kvbp.tile([128, D], BF, tag="vb", name=f"vt{gt}")
                for hf in range(2):
                    ps = psp.tile([128, 512], f32, tag="ps")
                    for dn in range(NDIN):
                        nc.tensor.matmul(
                            ps[:],
                            xts[dn][:, gt * 128:(gt + 1) * 128],
                            wv_sb[dn][:, hf * 512:(hf + 1) * 512],
                            start=(dn == 0), stop=(dn == NDIN - 1))
                    nc.vector.tensor_tensor(
                        vt[:, hf * 512:(hf + 1) * 512], ps[:],
                        bvr_sb[:, hf * 512:(hf + 1) * 512], ADD)
                return vt

            def kvacc(kvps, t, kt, vt):
                # pair p -> [128,128] block (both heads' diag) in bank p//4
                for p in range(NPAIR):
                    nc.tensor.matmul(
                        kvps[p // 4][:, (p % 4) * 128:(p % 4) * 128 + 128],
                        kt[:, p * 128:(p + 1) * 128],
                        vt[:, p * 128:(p + 1) * 128],
                        start=(t == 0 and p % 4 == 0),
                        stop=(t == NTB - 1 and p % 4 == 3))

            for b in range(B):
                kvps = [pskvp.tile([128, 512], f32, tag="kvps",
                                   name=f"kvps{b}_{w}") for w in range(2)]
                if b == 0:
                    # all 4 K tiles first: ~19us of PE work that only needs
                    # wk + the first x quarter, covering wv's DMA arrival
                    kts = [kproj(t) for t in range(NTB)]
                    for t in range(NTB):
                        kvacc(kvps, t, kts[t], vproj(t))
                else:
                    for t in range(NTB):
                        gt = b * NTB + t
                        kt = kproj(gt)
                        kvacc(kvps, t, kt, vproj(gt))
                # ship only the diagonal [64,64] blocks (head h = 2p+j)
                for p in range(NPAIR):
                    for j in range(2):
                        ex = kvexp.tile([64, 64], CCDT, tag="kvex",
                                        name=f"kvex{b}_{p}_{j}")
                        nc.vector.tensor_copy(
                            ex[:],
                            kvps[p // 4][j * 64:(j + 1) * 64,
                                         (p % 4) * 128 + j * 64:
                                         (p % 4) * 128 + j * 64 + 64])
                        h = 2 * p + j
                        nc.sync.dma_start(
                            bnc_in[b][h * 64:(h + 1) * 64, :], ex[:])
                nc.gpsimd.collective_compute(
                    "AllReduce", mybir.AluOpType.add,
                    replica_groups=[list(range(NCORES))],
                    ins=[bnc_in[b].opt()], outs=[bnc_out[b].opt()])

            # ---- Stage 2/3: Q^T proj, kv readout, fused o-proj ----
            kvsb = {}
            qts = {}

            def prefetch(b):
                for p in range(NPAIR):
                    kv = kvsbp.tile([128, 128], BF, tag="kvsb",
                                    name=f"kvsb{b}_{p}")
                    nc.vector.memset(kv[:], 0.0)
                    for j in range(2):
                        h = 2 * p + j
                        nc.sync.dma_start(
                            kv[j * 64:(j + 1) * 64, j * 64:(j + 1) * 64],
                            bnc_out[b][h * 64:(h + 1) * 64, :])
                    kvsb[(b, p)] = kv

            def qproj(b):
                for p in range(NPAIR):
                    ps = psp.tile([128, 512], f32, tag="ps")
                    for dn in range(NDIN):
                        nc.tensor.matmul(
                            ps[:],
                            wq_sb[dn][:, p * 128:(p + 1) * 128],
                            xts[dn][:, b * 512:(b + 1) * 512],
                            start=(dn == 0), stop=(dn == NDIN - 1))
                    qt = qtp.tile([128, 512], BF, tag="qt",
                                  name=f"qt{b}_{p}")
                    nc.scalar.activation(qt[:], ps[:], RELU,
                                         bias=bq_sb[:, p:p + 1])
                    qts[(b, p)] = qt

            def ro_oproj(b):
                otb = [otbp.tile([128, 512], BF, tag="otb",
                                 name=f"otb{b}_{p}") for p in range(NPAIR)]
                for p in range(NPAIR):
                    pso = psp.tile([128, 512], f32, tag="ps")
                    nc.tensor.matmul(pso[:], kvsb[(b, p)][:], qts[(b, p)][:],
                                     start=True, stop=True)
                    if p % 2 == 0:
                        nc.scalar.activation(otb[p][:], pso[:], COPY)
                    else:
                        nc.vector.tensor_copy(otb[p][:], pso[:])
                for t in range(NTB):
                    gt = b * NTB + t
                    for hf in range(2):
                        ps = psp.tile([128, 512], f32, tag="ps")
                        for dn in range(NDIN):
                            nc.tensor.matmul(
                                ps[:],
                                otb[dn][:, t * 128:(t + 1) * 128],
                                wo_sb[dn][:, hf * 512:(hf + 1) * 512],
                                start=(dn == 0), stop=(dn == NDIN - 1))
                        yt = ytp.tile([128, 512], BF, tag="yt")
                        nc.vector.tensor_tensor(
                            yt[:], ps[:],
                            bor_sb[:, hf * 512:(hf + 1) * 512], ADD)
                        nc.sync.dma_start(y_d[gt, hf], yt[:])

            prefetch(0)
            prefetch(1)
            prefetch(2)
            qproj(0)
            qproj(1)
            qproj(2)
            ro_oproj(0)
            prefetch(3)
            qproj(3)
            ro_oproj(1)
            ro_oproj(2)
            ro_oproj(3)

    nc.compile()
    _CACHE["nc_v6"] = nc
    return nc


def prepare_in_maps_v6(x, q_w, q_b, k_w, k_b, v_w, v_b, o_w, o_b):
    import ml_dtypes
    BF = ml_dtypes.bfloat16
    wqo = np.concatenate([q_w.T, o_w.T], axis=1)  # [D, 2D]
    bkvo = np.concatenate([
        np.broadcast_to(k_b.reshape(1, D), (128, D)),
        np.broadcast_to(v_b.reshape(1, D), (128, D)),
        np.broadcast_to(o_b.reshape(1, D), (128, D)),
    ], axis=1)  # [128, 3D]
    shared = {
        "wk": np.ascontiguousarray(k_w.T).astype(BF),
        "wv": np.ascontiguousarray(v_w.T).astype(BF),
        "wqo": np.ascontiguousarray(wqo).astype(BF),
        "bq": np.ascontiguousarray(q_b.reshape(NDIN, 128).T.astype(np.float32)),
        "bkvo": np.ascontiguousarray(bkvo).astype(BF),
    }
    in_maps = []
    for c in range(NCORES):
        xs = x[:, c * SC:(c + 1) * SC, :].reshape(T, D)
        xt = xs.T  # [D, T]
        m = dict(shared)
        # quarter-major: [4, D, 512] so each [128, 512] chunk is contiguous
        m["xt"] = np.ascontiguousarray(
            xt.reshape(D, 4, 512).transpose(1, 0, 2)).astype(BF)
        in_maps.append(m)
    return in_maps


def gather_output(results):
    y = np.empty((B, S, D), dtype=np.float32)
    for c in range(NCORES):
        # y_d is [NT, 2, 128, 512] -> [NT*128, 1024]
        yc = results[c]["y"].astype(np.float32)
        yc = yc.transpose(0, 2, 1, 3).reshape(T, D)
        y[:, c * SC:(c + 1) * SC, :] = yc.reshape(B, SC, D)
    return y


DTYPE = "v6"


def build_for(dtype):
    return build_program_v6()


def prepare_for(inputs, dtype):
    return prepare_in_maps_v6(**inputs)


def run(inputs, trace=False, dtype=None, **kw):
    from concourse import bass_utils
    dtype = dtype or DTYPE
    nc = build_for(dtype)
    in_maps = prepare_for(inputs, dtype)
    res = bass_utils.run_bass_kernel_spmd(
        nc, in_maps, core_ids=list(range(NCORES)), trace=trace, **kw)
    return gather_output(res.results), res


def kernel(**inputs):
    y, _ = run(inputs)
    return y


# revision 30
# speedup vs baseline: 1.0041x; 1.0041x over previous
"""Trainium2 Bass kernel for MinimalLinearAttention.

  q = relu(x @ q_w.T + q_b); k = relu(x @ k_w.T + k_b); v = x @ v_w.T + v_b
  kv[b,h] = sum_s k[b,s,h,:] outer v[b,s,h,:]          (per batch, all tokens)
  out[b,s,h] = q[b,s,h,:] @ kv[b,h]
  y = out @ o_w.T + o_b

Sharding: token-parallel over 8 cores. Each core takes a 512-token slice of
every batch (2048 tokens), computes k/v projections + partial kv, AllReduces
kv across cores (per batch), then does the q readout + output projection for
its own tokens. Host concatenates slices.

All-bf16 design, measured ~305us on HW (baseline 484us):
- Matmul operands bf16 so LDWEIGHTS (~105ns, FWL) hides under the ~263ns
  per-512-col matmul issue period (f32r loads take ~218ns and serialize).
  All weights stay resident in SBUF (12MB with x), so there is no
  stage-transition weight-load bubble.
- Input DMAs all on the sync queue (one HWDGE ring sustains ~360GB/s;
  splitting across two rings halves per-ring throughput), every transfer a
  contiguous DRAM block, ordered by first use: (wk[dn], x quarter-0 [dn])
  interleaved so the first K matmuls start at ~11us, then biases, wv,
  x q1, wq|wo merged per-din, x q2, q3.
- No bias matmuls: V/O biases ride the PSUM eviction as DVE tensor_tensor
  adds against host-replicated rows; K's bias is a DVE add then scalar-
  engine relu; Q bias is a per-partition activation scalar.
- kv pairs pack [128,128] blocks 4-to-a-bank (2 PSUM banks per batch),
  leaving 6 banks for projection-group rotation.
- Stage 2 hides the last kv AllReduce (~25us end-to-end; bounce-buffer
  reads wait on ALL collectives' semaphore): three batches of Q-projections
  (~50us of kv-independent PE work) run before the first readout, and
  readouts run long after their qt eviction so the PE never stalls on it.
- y stored bf16 in DRAM-contiguous [128,512] halves right after eviction.

On-device layouts (per core):
  xt   = x_slice.T            [4, D, 512] quarters (quarter = batch)
  w*   = W.T                  [Din=1024, Dout=1024] (wq|wo merged [D, 2048])
  K, V                        [T, D]     (from xt-stationary matmuls)
  Q^T                         [D, T]     (from w-stationary matmuls)
  kv per (batch, head-pair)   [128, 128] block-diagonal (2 heads of 64)
  y                           [NT, 2, 128, 512] bf16 (host reassembles f32)
"""

import os
import sys

os.environ.setdefault("MYCRO_LOCAL_CACHE", "1")

for _p in ("/opt/trn_rl_repo", "/root/.axon_site/_ro/trn_rl_repo"):
    if os.path.isdir(_p) and _p not in sys.path:
        sys.path.insert(0, _p)

import numpy as np

B, S, D, H, HD = 4, 4096, 1024, 16, 64
NCORES = 8
SC = S // NCORES          # 512 tokens per core per batch
T = B * SC                # 2048 rows per core
NPAIR = 8                 # head pairs (2 heads of 64 dims = 128 partitions)
NDIN = D // 128           # 8 Din tiles
NT = T // 128             # 16 T tiles per core
NTB = SC // 128           # 4 T tiles per batch

CC_BF16 = True            # bf16 kv collective payload

_CACHE = {}


def build_program_v6():
    if "nc_v6" in _CACHE:
        return _CACHE["nc_v6"]

    import concourse.bacc as bacc
    import concourse.tile as tile
    from concourse import bass, mybir

    f32 = mybir.dt.float32
    BF = mybir.dt.bfloat16
    CCDT = BF if CC_BF16 else f32
    RELU = mybir.ActivationFunctionType.Relu
    COPY = mybir.ActivationFunctionType.Copy
    ADD = mybir.AluOpType.add

    nc = bacc.Bacc("TRN2", target_bir_lowering=False, debug=False,
                   num_devices=NCORES)

    xt_d = nc.dram_tensor("xt", [4, D, 512], BF, kind="ExternalInput").ap()
    wk_d = nc.dram_tensor("wk", [D, D], BF, kind="ExternalInput").ap()
    wv_d = nc.dram_tensor("wv", [D, D], BF, kind="ExternalInput").ap()
    wqo_d = nc.dram_tensor("wqo", [D, 2 * D], BF, kind="ExternalInput").ap()
    bq_d = nc.dram_tensor("bq", [128, NDIN], f32, kind="ExternalInput").ap()
    # bkvo = [bkr | bvr | bor], each [128, D] replicated rows
    bkvo_d = nc.dram_tensor("bkvo", [128, 3 * D], BF,
                            kind="ExternalInput").ap()
    y_d = nc.dram_tensor("y", [NT, 2, 128, 512], BF,
                         kind="ExternalOutput").ap()

    HPB = 16 * 64  # bounce rows per batch: 16 heads x 64 d-rows

    with tile.TileContext(nc) as tc:
        with (
            tc.tile_pool(name="const", bufs=1) as constp,
            tc.tile_pool(name="wp", bufs=1) as wp,
            tc.tile_pool(name="xtp", bufs=1) as xtp,
            tc.tile_pool(name="kvb", bufs=6) as kvbp,
            tc.tile_pool(name="ktmp", bufs=3) as ktmpp,
            tc.tile_pool(name="qt", bufs=24) as qtp,
            tc.tile_pool(name="otb", bufs=10) as otbp,
            tc.tile_pool(name="kvex", bufs=8) as kvexp,
            tc.tile_pool(name="kvsb", bufs=24) as kvsbp,
            tc.tile_pool(name="yt", bufs=4) as ytp,
            tc.tile_pool(name="dram", bufs=1, space="DRAM") as dramp,
            tc.tile_pool(name="ps", bufs=6, space="PSUM") as psp,
            tc.tile_pool(name="pskv", bufs=2, space="PSUM") as pskvp,
        ):
            # ---- loads: first-use order, all on the sync queue (a single
            # HWDGE ring sustains ~360GB/s; splitting across two rings
            # halves per-ring throughput) ----
            qs = [nc.sync, nc.scalar]

            def dma(dst, src):
                nc.sync.dma_start(dst, src)

            wk_sb = [wp.tile([128, D], BF, tag=f"wk{dn}", name=f"wk_sb{dn}")
                     for dn in range(NDIN)]
            wv_sb = [wp.tile([128, D], BF, tag=f"wv{dn}", name=f"wv_sb{dn}")
                     for dn in range(NDIN)]
            xts = [xtp.tile([128, T], BF, tag=f"xt{dn}", name=f"xt_sb{dn}")
                   for dn in range(NDIN)]
            bkvo_sb = constp.tile([128, 3 * D], BF, tag="bkvo")
            bkr_sb = bkvo_sb[:, 0:D]
            bvr_sb = bkvo_sb[:, D:2 * D]
            bor_sb = bkvo_sb[:, 2 * D:3 * D]

            def xq(dn, q):
                return (xts[dn][:, q * 512:(q + 1) * 512],
                        xt_d[q, dn * 128:(dn + 1) * 128, :])

            # interleave wk[dn] with its x chunk so matmul dn of the first
            # K group starts as soon as its own operands land
            for dn in range(NDIN):
                dma(wk_sb[dn][:], wk_d[dn * 128:(dn + 1) * 128, :])
                dma(*xq(dn, 0))
            # biases early: the K eviction adds bkr; PSUM recycling depends
            # on those evictions
            dma(bkvo_sb[:], bkvo_d[:])
            for dn in range(NDIN):
                dma(wv_sb[dn][:], wv_d[dn * 128:(dn + 1) * 128, :])
            for dn in range(NDIN):
                dma(*xq(dn, 1))
            wqo_sb = [wp.tile([128, 2 * D], BF, tag=f"wqo{dn}",
                              name=f"wqo_sb{dn}") for dn in range(NDIN)]
            for dn in range(NDIN):
                dma(wqo_sb[dn][:], wqo_d[dn * 128:(dn + 1) * 128, :])
            for dn in range(NDIN):
                dma(*xq(dn, 2))
            for dn in range(NDIN):
                dma(*xq(dn, 3))
            wq_sb = [wqo_sb[dn][:, 0:D] for dn in range(NDIN)]
            wo_sb = [wqo_sb[dn][:, D:2 * D] for dn in range(NDIN)]
            bq_sb = constp.tile([128, NDIN], f32, tag="bq")
            dma(bq_sb[:], bq_d[:])

            bnc_in = [dramp.tile([HPB, 64], CCDT, tag=f"bi{b}",
                                 name=f"bnc_in{b}") for b in range(B)]
            bnc_out = [dramp.tile([HPB, 64], CCDT, tag=f"bo{b}",
                                  addr_space="Shared", name=f"bnc_out{b}")
                       for b in range(B)]

            # ---- Stage 1: K,V projections + per-batch partial kv ----
            def kproj(gt):
                kt = kvbp.tile([128, D], BF, tag="kb", name=f"kt{gt}")
                for hf in range(2):
                    ps = psp.tile([128, 512], f32, tag="ps")
                    for dn in range(NDIN):
                        nc.tensor.matmul(
                            ps[:],
                            xts[dn][:, gt * 128:(gt + 1) * 128],
                            wk_sb[dn][:, hf * 512:(hf + 1) * 512],
                            start=(dn == 0), stop=(dn == NDIN - 1))
                    ktmp = ktmpp.tile([128, 512], BF, tag="ktmp")
                    nc.vector.tensor_tensor(
                        ktmp[:], ps[:],
                        bkr_sb[:, hf * 512:(hf + 1) * 512], ADD)
                    nc.scalar.activation(
                        kt[:, hf * 512:(hf + 1) * 512], ktmp[:], RELU)
                return kt

            def vproj(gt):
                vt = kvbp.tile([128, D], BF, tag="vb", name=f"vt{gt}")
                for hf in range(2):
                    ps = psp.tile([128, 512], f32, tag="ps")
                    for dn in range(NDIN):
                        nc.tensor.matmul(
                            ps[:],
                            xts[dn][:, gt * 128:(gt + 1) * 128],
                            wv_sb[dn][:, hf * 512:(hf + 1) * 512],
                            start=(dn == 0), stop=(dn == NDIN - 1))
                    nc.vector.tensor_tensor(
                        vt[:, hf * 512:(hf + 1) * 512], ps[:],
                        bvr_sb[:, hf * 512:(hf + 1) * 512], ADD)
                return vt

            def kvacc(kvps, t, kt, vt):
                # pair p -> [128,128] block (both heads' diag) in bank p//4
                for p in range(NPAIR):
                    nc.tensor.matmul(
                        kvps[p // 4][:, (p % 4) * 128:(p % 4) * 128 + 128],
                        kt[:, p * 128:(p + 1) * 128],
                        vt[:, p * 128:(p + 1) * 128],
                        start=(t == 0 and p % 4 == 0),
                        stop=(t == NTB - 1 and p % 4 == 3))

            for b in range(B):
                kvps = [pskvp.tile([128, 512], f32, tag="kvps",
                                   name=f"kvps{b}_{w}") for w in range(2)]
                if b == 0:
                    # all 4 K tiles first: ~19us of PE work that only needs
                    # wk + the first x quarter, covering wv's DMA arrival
                    kts = [kproj(t) for t in range(NTB)]
                    for t in range(NTB):
                        kvacc(kvps, t, kts[t], vproj(t))
                else:
                    for t in range(NTB):
                        gt = b * NTB + t
                        kt = kproj(gt)
                        kvacc(kvps, t, kt, vproj(gt))
                # ship only the diagonal [64,64] blocks (head h = 2p+j)
                for p in range(NPAIR):
                    for j in range(2):
                        ex = kvexp.tile([64, 64], CCDT, tag="kvex",
                                        name=f"kvex{b}_{p}_{j}")
                        nc.vector.tensor_copy(
                            ex[:],
                            kvps[p // 4][j * 64:(j + 1) * 64,
                                         (p % 4) * 128 + j * 64:
                                         (p % 4) * 128 + j * 64 + 64])
                        h = 2 * p + j
                        nc.sync.dma_start(
                            bnc_in[b][h * 64:(h + 1) * 64, :], ex[:])
                nc.gpsimd.collective_compute(
                    "AllReduce", mybir.AluOpType.add,
                    replica_groups=[list(range(NCORES))],
                    ins=[bnc_in[b].opt()], outs=[bnc_out[b].opt()])

            # ---- Stage 2/3: Q^T proj, kv readout, fused o-proj ----
            kvsb = {}
            qts = {}

            def prefetch(b):
                for p in range(NPAIR):
                    kv = kvsbp.tile([128, 128], BF, tag="kvsb",
                                    name=f"kvsb{b}_{p}")
                    nc.vector.memset(kv[:], 0.0)
                    for j in range(2):
                        h = 2 * p + j
                        nc.sync.dma_start(
                            kv[j * 64:(j + 1) * 64, j * 64:(j + 1) * 64],
                            bnc_out[b][h * 64:(h + 1) * 64, :])
                    kvsb[(b, p)] = kv

            def qproj(b):
                for p in range(NPAIR):
                    ps = psp.tile([128, 512], f32, tag="ps")
                    for dn in range(NDIN):
                        nc.tensor.matmul(
                            ps[:],
                            wq_sb[dn][:, p * 128:(p + 1) * 128],
                            xts[dn][:, b * 512:(b + 1) * 512],
                            start=(dn == 0), stop=(dn == NDIN - 1))
                    qt = qtp.tile([128, 512], BF, tag="qt",
                                  name=f"qt{b}_{p}")
                    nc.scalar.activation(qt[:], ps[:], RELU,
                                         bias=bq_sb[:, p:p + 1])
                    qts[(b, p)] = qt

            def ro_oproj(b):
                otb = [otbp.tile([128, 512], BF, tag="otb",
                                 name=f"otb{b}_{p}") for p in range(NPAIR)]
                for p in range(NPAIR):
                    pso = psp.tile([128, 512], f32, tag="ps")
                    nc.tensor.matmul(pso[:], kvsb[(b, p)][:], qts[(b, p)][:],
                                     start=True, stop=True)
                    if p % 2 == 0:
                        nc.scalar.activation(otb[p][:], pso[:], COPY)
                    else:
                        nc.vector.tensor_copy(otb[p][:], pso[:])
                for t in range(NTB):
                    gt = b * NTB + t
                    for hf in range(2):
                        ps = psp.tile([128, 512], f32, tag="ps")
                        for dn in range(NDIN):
                            nc.tensor.matmul(
                                ps[:],
                                otb[dn][:, t * 128:(t + 1) * 128],
                                wo_sb[dn][:, hf * 512:(hf + 1) * 512],
                                start=(dn == 0), stop=(dn == NDIN - 1))
                        yt = ytp.tile([128, 512], BF, tag="yt")
                        nc.vector.tensor_tensor(
                            yt[:], ps[:],
                            bor_sb[:, hf * 512:(hf + 1) * 512], ADD)
                        nc.sync.dma_start(y_d[gt, hf], yt[:])

            prefetch(0)
            prefetch(1)
            prefetch(2)
            qproj(0)
            qproj(1)
            qproj(2)
            ro_oproj(0)
            prefetch(3)
            qproj(3)
            ro_oproj(1)
            ro_oproj(2)
            ro_oproj(3)

    nc.compile()
    _CACHE["nc_v6"] = nc
    return nc


def prepare_in_maps_v6(x, q_w, q_b, k_w, k_b, v_w, v_b, o_w, o_b):
    import ml_dtypes
    BF = ml_dtypes.bfloat16
    wqo = np.concatenate([q_w.T, o_w.T], axis=1)  # [D, 2D]
    bkvo = np.concatenate([
        np.broadcast_to(k_b.reshape(1, D), (128, D)),
        np.broadcast_to(v_b.reshape(1, D), (128, D)),
        np.broadcast_to(o_b.reshape(1, D), (128, D)),
    ], axis=1)  # [128, 3D]
    shared = {
        "wk": np.ascontiguousarray(k_w.T).astype(BF),
        "wv": np.ascontiguousarray(v_w.T).astype(BF),
        "wqo": np.ascontiguousarray(wqo).astype(BF),
        "bq": np.ascontiguousarray(q_b.reshape(NDIN, 128).T.astype(np.float32)),
        "bkvo": np.ascontiguousarray(bkvo).astype(BF),
    }
    in_maps = []
    for c in range(NCORES):
        xs = x[:, c * SC:(c + 1) * SC, :].reshape(T, D)
        xt = xs.T  # [D, T]
        m = dict(shared)
        # quarter-major: [4, D, 512] so each [128, 512] chunk is contiguous
        m["xt"] = np.ascontiguousarray(
            xt.reshape(D, 4, 512).transpose(1, 0, 2)).astype(BF)
        in_maps.append(m)
    return in_maps


def gather_output(results):
    y = np.empty((B, S, D), dtype=np.float32)
    for c in range(NCORES):
        # y_d is [NT, 2, 128, 512] -> [NT*128, 1024]
        yc = results[c]["y"].astype(np.float32)
        yc = yc.transpose(0, 2, 1, 3).reshape(T, D)
        y[:, c * SC:(c + 1) * SC, :] = yc.reshape(B, SC, D)
    return y


DTYPE = "v6"


def build_for(dtype):
    return build_program_v6()


def prepare_for(inputs, dtype):
    return prepare_in_maps_v6(**inputs)


def run(inputs, trace=False, dtype=None, **kw):
    from concourse import bass_utils
    dtype = dtype or DTYPE
    nc = build_for(dtype)
    in_maps = prepare_for(inputs, dtype)
    res = bass_utils.run_bass_kernel_spmd(
        nc, in_maps, core_ids=list(range(NCORES)), trace=trace, **kw)
    return gather_output(res.results), res


def kernel(**inputs):
    inputs = {k: np.asarray(v) for k, v in inputs.items()}
    y, _ = run(inputs)
    return y
